# revision 2
# baseline (speedup 1.0000x reference)
"""Multi-head attention (B=8, S=2048, D=512, H=8, DH=64) on 8 TRN2 NeuronCores.

Strategy: data-parallel over the batch dim — core b computes batch element b
end-to-end (no collectives). Per core, everything is kept transposed
("feature on partitions") so softmax reductions land on the TensorE
contraction axis:

  1. QKV projection with head-interleaved, pre-transposed weights gives
     Q^T, K^T laid out (64h+c, s) and V laid out (s, 64h+c). K^T is
     pre-scaled by SCALE*log2(e) on the host so the score matmuls produce
     log2-domain logits directly.
  2. Scores are computed transposed, S^T[j, i], as K=64 matmuls row-packed
     two-at-a-time into disjoint PE row groups (lo/hi replicas of Q^T/K^T).
  3. 2^y runs out of PSUM into bf16 SBUF, split across TWO engines: most
     [128,1024] units go to ScalarE (func=Exp, scale=ln2); ~1/4 go to the
     DVE via two custom ops (EXP2_PREP_ANT builds the 2^n * c^2 exponent
     word with an int32 write-conversion Schraudolph trick; EXP2_FIX_ANT
     applies the (f + d/c)^2 mantissa correction). The DVE path has ~2.2%
     pointwise error which washes out in the softmax ratio (<1e-3 on the
     final output).
  4. O^T[c, i] = sum_j Vaug[j, c] E^T[j, i] with Vaug = [V | ones]: M=65
     matmuls whose 65th row accumulates the softmax denominator for free.
  5. Normalization: O_un is copied out of PSUM, the denominator row is
     reshaped to (128,16) for a cheap reciprocal, round-trips through
     DRAM for a partition-broadcast, and a GpSimd multiply (VectorE for
     the last head, which is tail-critical) writes the normalized O^T.
  6. Only QK chunk 0 + V precede head 0; QK chunks 1-3 are emitted between
     heads so ScalarE/DVE start exp ~40us earlier.
  7. The output projection (+bias) produces out^T which the host
     transposes back.
"""

import numpy as np
import ml_dtypes

B, S, D = 8, 2048, 512
H, DH = 8, 64
INNER = H * DH
SCALE = DH ** -0.5
LOG2E = 1.4426950408889634
LN2 = 0.6931471805599453

N_CORES = 8
NDT = D // 128   # 4 contraction tiles
NSC = S // 128   # 16 s-chunks (j-chunks)
NST = S // 512   # 4 s-tiles

# ---- custom DVE exp2 constants ----
C_FIT, D_FIT = 0.34690329774974804, 1.0109504947068022
_C2F = C_FIT * C_FIT
_E2 = -4                                     # floor(log2(c^2))
_M2 = (_C2F / 2.0 ** _E2 - 1.0) * 2.0 ** 23
C2_VAL = float((127 + _E2) * 2 ** 23 + _M2)  # exponent bias + c^2 mantissa
CM = 12582912.0                              # 1.5 * 2^23 round-to-int magic
DC = D_FIT / C_FIT

# Exp unit assignment: units indexed by (head*16 + chunk)*2 + half; DVE when
# idx % 15 lands in DVE_SLOTS (~27% of 256 units).
DVE_SLOTS = (3, 7, 11, 14)


def _is_dve_unit(idx):
    return (idx % 15) in DVE_SLOTS


def register_exp2_ops():
    from concourse import dve_ops
    from concourse.dve_spec import Spec, Src0, Src1, C0, C1, C2, sq, lower
    from concourse.dve_spec import _has_src1
    from concourse.dve_uop import DveOpSpec

    if "EXP2_PREP_ANT" in dve_ops.CUSTOM_DVE_SPECS:
        by = {op.name: op for op in dve_ops.OPS}
        return by["EXP2_PREP_ANT"], by["EXP2_FIX_ANT"]

    f32 = np.float32

    def ref_prep(in0, in1, s0, s1, imm2):
        t = (in0.astype(f32) + f32(s0)).astype(f32)
        n = (t - f32(s0)).astype(f32)
        return ((n * f32(s1)).astype(f32) + f32(imm2)).astype(f32)

    def ref_fix(in0, in1, s0, s1, imm2):
        t = (in0.astype(f32) + f32(s0)).astype(f32)
        n = (t - f32(s0)).astype(f32)
        g = ((in0.astype(f32) - n).astype(f32) + f32(s1)).astype(f32)
        return ((g * g).astype(f32) * in1.astype(f32)).astype(f32)

    spec_prep = Spec(body=((Src0 + C0) - C0) * C1 + C2, reference=ref_prep)
    spec_fix = Spec(body=sq((Src0 - ((Src0 + C0) - C0)) + C1) * Src1,
                    reference=ref_fix)

    ops = []
    for name, spec in [("EXP2_PREP_ANT", spec_prep), ("EXP2_FIX_ANT", spec_fix)]:
        row = dve_ops._CUSTOM_DVE_ROW_BASE + len(dve_ops.OPS)
        assert row < 0x20
        dve_ops._SUB_OPCODE_FOR_NAME[name] = row
        tmp = DveOpSpec(name=name, opcode=row, uops=lower(spec, ver="v3"),
                        rd1_en=_has_src1(spec))
        op = dve_ops.DveOp(name, spec, subdim=False,
                           uops_sha={"v3": tmp.sha("v3")})
        dve_ops.OPS.append(op)
        dve_ops.CUSTOM_DVE_SPECS[name] = spec
        ops.append(op)
    return ops


def _build_kernel():
    import concourse.bass as bass
    import concourse.mybir as mybir
    import concourse.tile as tile
    from concourse import bacc

    OP_PREP, OP_FIX = register_exp2_ops()

    bf16 = mybir.dt.bfloat16
    f32 = mybir.dt.float32
    i32 = mybir.dt.int32
    Exp = mybir.ActivationFunctionType.Exp

    nc = bacc.Bacc()

    xT = nc.declare_dram_parameter("xT", [D, S], bf16, isOutput=False)
    wq = nc.declare_dram_parameter("wq", [D, INNER], bf16, isOutput=False)
    wk = nc.declare_dram_parameter("wk", [D, INNER], bf16, isOutput=False)
    wv = nc.declare_dram_parameter("wv", [D, INNER], bf16, isOutput=False)
    wo = nc.declare_dram_parameter("wo", [INNER, D], bf16, isOutput=False)
    bo = nc.declare_dram_parameter("bo", [NDT, 128, 1], f32, isOutput=False)
    out = nc.declare_dram_parameter("out", [D, S], f32, isOutput=True)
    den_dram = nc.dram_tensor("den_scratch", [H, S], f32)

    with tile.TileContext(nc) as tc:
        with (
            tc.tile_pool(name="weights", bufs=1) as wpool,
            tc.tile_pool(name="acts", bufs=1) as apool,
            tc.tile_pool(name="et", bufs=3) as epool,
            tc.tile_pool(name="u0", bufs=2) as upool,
            tc.tile_pool(name="small", bufs=2) as spool,
            tc.tile_pool(name="ostage", bufs=2) as opool,
            tc.tile_pool(name="psA", bufs=2, space="PSUM") as psA,
            tc.tile_pool(name="psV", bufs=1, space="PSUM") as psV,
        ):
            # ---- load inputs (x and q/k weights first: they gate head 0) ----
            xT_s = [[wpool.tile([128, S // 2], bf16, name=f"xT{d}_{hf}",
                              tag=f"xT{d}_{hf}") for hf in range(2)]
                    for d in range(NDT)]
            wq_s = [wpool.tile([128, INNER], bf16, name=f"wq{d}", tag=f"wq{d}")
                    for d in range(NDT)]
            wk_s = [wpool.tile([128, INNER], bf16, name=f"wk{d}", tag=f"wk{d}")
                    for d in range(NDT)]
            wv_s = [wpool.tile([128, INNER], bf16, name=f"wv{d}", tag=f"wv{d}")
                    for d in range(NDT)]
            wo_s = [wpool.tile([128, D], bf16, name=f"wo{d}", tag=f"wo{d}")
                    for d in range(NDT)]
            bo_s = [wpool.tile([128, 1], f32, name=f"bo{d}", tag=f"bo{d}")
                    for d in range(NDT)]
            for d in range(NDT):
                sl = slice(d * 128, (d + 1) * 128)
                nc.sync.dma_start(out=xT_s[d][0][:], in_=xT[sl, 0:S // 2])
                nc.sync.dma_start(out=wq_s[d][:], in_=wq[sl, :])
                nc.sync.dma_start(out=wk_s[d][:], in_=wk[sl, :])
            for d in range(NDT):
                sl = slice(d * 128, (d + 1) * 128)
                nc.scalar.dma_start(out=xT_s[d][1][:], in_=xT[sl, S // 2:])
            for d in range(NDT):
                sl = slice(d * 128, (d + 1) * 128)
                nc.scalar.dma_start(out=wv_s[d][:], in_=wv[sl, :])
                nc.scalar.dma_start(out=wo_s[d][:], in_=wo[sl, :])
                nc.scalar.dma_start(out=bo_s[d][:], in_=bo[d, :, :])

            # ---- QKV projection ----
            qt_lo = [apool.tile([128, S], bf16, name=f"qlo{t}", tag=f"qlo{t}")
                     for t in range(NDT)]
            kt_lo = [apool.tile([128, S], bf16, name=f"klo{t}", tag=f"klo{t}")
                     for t in range(NDT)]
            qt_hi = [apool.tile([128, S], bf16, name=f"qhi{t}", tag=f"qhi{t}")
                     for t in range(NDT)]
            kt_hi = [apool.tile([128, S], bf16, name=f"khi{t}", tag=f"khi{t}")
                     for t in range(NDT)]

            # PE warm-up: junk matmuls during the input-DMA window keep the
            # HAM activity monitor busy so real matmuls start at 2.4 GHz.
            junk_sb = wpool.tile([128, 512], bf16, name="junk", tag="junk")
            nc.vector.memset(junk_sb[:, :], 0.0)
            junk_ps = psV.tile([128, 4 * 512], f32, name="junkps", tag="pv")
            for k in range(16):
                nc.tensor.matmul(
                    junk_ps[:, (k % 4) * 512:(k % 4 + 1) * 512],
                    lhsT=junk_sb[:, 0:128],
                    rhs=junk_sb[:, :],
                )

            def qk_chunk(w_s, dst, ch):
                for half in range(2):  # s in 1024-halves
                    pa = psA.tile([128, 1024], f32, name="pa", tag="pa")
                    for d in range(NDT):
                        for nn in range(2):
                            s0 = nn * 512
                            nc.tensor.matmul(
                                pa[:, nn * 512:(nn + 1) * 512],
                                lhsT=w_s[d][:, ch * 128:(ch + 1) * 128],
                                rhs=xT_s[d][half][:, s0:s0 + 512],
                                start=(d == 0),
                                stop=(d == NDT - 1),
                            )
                    nc.vector.tensor_copy(
                        dst[ch][:, half * 1024:(half + 1) * 1024], pa[:, :])

            def swap_halves(t):
                for (lo, hi) in ((qt_lo, qt_hi), (kt_lo, kt_hi)):
                    nc.sync.dma_start(out=hi[t][64:128, :], in_=lo[t][0:64, :])
                    nc.sync.dma_start(out=hi[t][0:64, :], in_=lo[t][64:128, :])

            qk_chunk(wq_s, qt_lo, 0)
            qk_chunk(wk_s, kt_lo, 0)
            swap_halves(0)

            v_aug = [apool.tile([128, H * (DH + 1)], bf16, name=f"va{m}",
                                tag=f"va{m}") for m in range(NSC)]

            def v_round(r):
                pvt = psV.tile([128, 4 * 512], f32, name="pvt", tag="pv")
                for k in range(4):
                    m = 4 * r + k
                    for d in range(NDT):
                        mh, mo = divmod(m, 8)
                        nc.tensor.matmul(
                            pvt[:, k * 512:(k + 1) * 512],
                            lhsT=xT_s[d][mh][:, mo * 128:(mo + 1) * 128],
                            rhs=wv_s[d][:, :],
                            start=(d == 0),
                            stop=(d == NDT - 1),
                        )
                for k in range(4):
                    m = 4 * r + k
                    va = v_aug[m].rearrange("p (h t) -> p h t", t=DH + 1)
                    nc.vector.tensor_copy(
                        va[:, :, 0:DH],
                        pvt[:, k * 512:(k + 1) * 512].rearrange(
                            "p (h t) -> p h t", t=DH),
                    )
                    nc.vector.memset(va[:, :, DH:DH + 1], 1.0)

            for r in range(NSC // 4):
                v_round(r)

            # ---- attention, head by head; remaining QK chunks interleave ----
            ot = [apool.tile([128, S], bf16, name=f"ot{t}", tag=f"ot{t}")
                  for t in range(NDT)]
            f32d = f32

            def head(h):
                t, p = h // 2, h % 2
                lo_sl = slice(64 * p, 64 * p + 64)
                hi_sl = slice(64 * (1 - p), 64 * (1 - p) + 64)
                pv = psV.tile([128, 4 * 512], f32, name="pvh", tag="pv")
                ets = {}

                def pv_mms(jc):
                    for it in range(NST):
                        nc.tensor.matmul(
                            pv[0:DH + 1, it * 512:(it + 1) * 512],
                            lhsT=v_aug[jc][:, h * (DH + 1):(h + 1) * (DH + 1)],
                            rhs=ets[jc][:, it * 512:(it + 1) * 512],
                            start=(jc == 0),
                            stop=(jc == NSC - 1),
                        )

                trail = 1
                for jc in range(NSC):
                    et = epool.tile([128, S], bf16, name="et", tag="et")
                    ets[jc] = et
                    for half in range(2):
                        pa = psA.tile([128, 1024], f32, name="pa", tag="pa")
                        i0, i1 = 2 * half, 2 * half + 1
                        nc.tensor.matmul(
                            pa[:, 0:512],
                            lhsT=kt_lo[t][lo_sl, jc * 128:(jc + 1) * 128],
                            rhs=qt_lo[t][lo_sl, i0 * 512:(i0 + 1) * 512],
                        )
                        nc.tensor.matmul(
                            pa[:, 512:1024],
                            lhsT=kt_hi[t][hi_sl, jc * 128:(jc + 1) * 128],
                            rhs=qt_hi[t][hi_sl, i1 * 512:(i1 + 1) * 512],
                        )
                        uidx = (h * NSC + jc) * 2 + half
                        dst = et[:, half * 1024:(half + 1) * 1024]
                        if _is_dve_unit(uidx):
                            u0 = upool.tile([128, 1024], i32, name="u0",
                                            tag="u0")
                            nc.vector._custom_dve(
                                OP_PREP, out=u0[:], in0=pa[:, :],
                                s0=CM, s1=8388608.0, imm2=C2_VAL)
                            nc.vector._custom_dve(
                                OP_FIX, out=dst, in0=pa[:, :],
                                in1=u0[:].bitcast(f32d), s0=CM, s1=DC)
                        else:
                            nc.scalar.activation(
                                out=dst, in_=pa[:, :], func=Exp, scale=LN2)
                    if jc >= trail:
                        pv_mms(jc - trail)
                for jc in range(NSC - trail, NSC):
                    pv_mms(jc)

                # Decouple normalization from the PE pipeline: get O_un and
                # the denominator row out of PSUM fast, then normalize via
                # a cheap (128,16) reciprocal + DRAM partition-broadcast.
                oun = spool.tile([DH + 1, S], f32, name="oun", tag="oun")
                nc.vector.tensor_copy(oun[:, :], pv[0:DH + 1, :])
                den128 = spool.tile([128, 16], f32, name="den128", tag="d128")
                nc.sync.dma_start(out=den128[:, :], in_=oun[DH:DH + 1, :])
                nc.vector.reciprocal(out=den128[:, :], in_=den128[:, :])
                nc.sync.dma_start(out=den_dram[h, :], in_=den128[:, :])
                bc = spool.tile([64, S], f32, name="bc", tag="bc")
                dd = den_dram[h:h + 1, :]
                bcast_src = bass.AP(
                    tensor=dd.tensor,
                    offset=dd.offset,
                    ap=[[0, 64]] + [list(x) for x in dd.ap[1:]],
                )
                nc.sync.dma_start(out=bc[:, :], in_=bcast_src)
                norm_eng = nc.gpsimd if h < H - 1 else nc.vector
                norm_eng.tensor_mul(
                    ot[t][64 * p:64 * p + 64, :], oun[0:DH, :], bc[:, :])

            for h in range(H):
                head(h)
                if h < NDT - 1:
                    qk_chunk(wq_s, qt_lo, h + 1)
                    qk_chunk(wk_s, kt_lo, h + 1)
                    swap_halves(h + 1)

            # ---- output projection (psA ping-pong so matmul groups and the
            # bias-add/copy of the previous group overlap) ----
            for ch in range(NDT):
                stage = opool.tile([128, S], f32, name="stage", tag="stage")
                for half in range(2):
                    po = psA.tile([128, 1024], f32, name="pa", tag="pa")
                    for st2 in range(2):
                        st = half * 2 + st2
                        for kt in range(NDT):
                            nc.tensor.matmul(
                                po[:, st2 * 512:(st2 + 1) * 512],
                                lhsT=wo_s[kt][:, ch * 128:(ch + 1) * 128],
                                rhs=ot[kt][:, st * 512:(st + 1) * 512],
                                start=(kt == 0),
                                stop=(kt == NDT - 1),
                            )
                    nc.vector.tensor_scalar_add(
                        out=stage[:, half * 1024:(half + 1) * 1024],
                        in0=po[:, :],
                        scalar1=bo_s[ch][:, :],
                    )
                    nc.sync.dma_start(
                        out=out[ch * 128:(ch + 1) * 128,
                                half * 1024:(half + 1) * 1024],
                        in_=stage[:, half * 1024:(half + 1) * 1024],
                    )

    nc.finalize()
    return nc


_NC_CACHE = None


def _get_nc():
    global _NC_CACHE
    if _NC_CACHE is None:
        _NC_CACHE = _build_kernel()
    return _NC_CACHE


def _prep_inputs(x, W_qkv, W_out, b_out):
    bf16 = ml_dtypes.bfloat16
    # head-interleave and transpose the qkv weight: row 192h+{0,64,128}+c of
    # W_qkv is q/k/v row (h, c); regroup to e' = 64h+c and transpose to [d, e']
    w3 = W_qkv.reshape(H, 3, DH, D)
    wq_h = np.ascontiguousarray(w3[:, 0].reshape(INNER, D).T).astype(bf16)
    # fold softmax scale and log2(e) into K so scores are log2-domain logits
    wk_h = np.ascontiguousarray(
        w3[:, 1].reshape(INNER, D).T * np.float32(SCALE * LOG2E)).astype(bf16)
    wv_h = np.ascontiguousarray(w3[:, 2].reshape(INNER, D).T).astype(bf16)
    wo_h = np.ascontiguousarray(W_out.T).astype(bf16)  # [hc, d]
    bo_h = np.ascontiguousarray(b_out.reshape(NDT, 128, 1)).astype(np.float32)
    in_maps = []
    for b in range(N_CORES):
        xT_b = np.ascontiguousarray(x[b].T).astype(bf16)  # [d, s]
        in_maps.append({
            "xT": xT_b, "wq": wq_h, "wk": wk_h, "wv": wv_h,
            "wo": wo_h, "bo": bo_h,
        })
    return in_maps


def kernel(x, W_qkv, W_out, b_out):
    from concourse.bass_utils import run_bass_kernel_spmd

    in_maps = _prep_inputs(x, W_qkv, W_out, b_out)
    nc = _get_nc()
    res = run_bass_kernel_spmd(nc, in_maps, list(range(N_CORES)))
    outs = [res.results[b]["out"].T for b in range(N_CORES)]  # [s, d] each
    return np.ascontiguousarray(np.stack(outs, axis=0)).astype(np.float32)
